# revision 4
# baseline (speedup 1.0000x reference)
"""KLDivLoss(batchmean) of softmax(f1_rewards/tau) against log(output).

Contract: kernel(output=[1024,4096,1] f32, labels=[1024,4096] i32) -> () f32.

Math (per batch row):
    c_k  = cumsum(labels)            (k = 1..L)
    T    = c_L
    r_k  = 2*c_k / (k + T)           (== F1@k; the where() guards in the
                                      reference collapse since c_k = 0 => r_k = 0)
    s_k  = r_k / tau                 (s in [0, 2/(2*tau)] ~ [0, 1.18] -> exp safe
                                      without max-subtraction)
    q    = softmax(s);  Z = sum exp(s);  log q = s - ln Z
    row  = sum_k q_k*s_k - ln Z - sum_k q_k*ln p_k
    loss = sum_rows(row) / B

Distribution: pure data-parallel, 128 batch rows per NeuronCore (= the 128
SBUF partitions), 8 cores. Each core emits one f32 partial (its row-sum);
the host adds the 8 partials and divides by B.
"""

import numpy as np

B, L = 1024, 4096
N_CORES = 8
RPC = B // N_CORES  # rows per core = 128 = SBUF partitions
TAU = 0.85
CH = 512  # free-dim chunk
NCH = L // CH

_NC_CACHE = {}


def build_nc():
    import concourse.bacc as bacc
    import concourse.bass_isa as bass_isa
    import concourse.mybir as mybir
    import concourse.tile as tile

    f32 = mybir.dt.float32
    i32 = mybir.dt.int32
    Alu = mybir.AluOpType
    Act = mybir.ActivationFunctionType
    Ax = mybir.AxisListType

    nc = bacc.Bacc(
        "TRN2", target_bir_lowering=False, debug=False, num_devices=N_CORES
    )
    labels_d = nc.dram_tensor("labels", [RPC, L], i32, kind="ExternalInput").ap()
    p_d = nc.dram_tensor("p", [RPC, L], f32, kind="ExternalInput").ap()
    out_d = nc.dram_tensor("partial", [1, 1], f32, kind="ExternalOutput").ap()

    with tile.TileContext(nc) as tc:
        with (
            tc.tile_pool(name="persist", bufs=1) as persist,
            tc.tile_pool(name="lab", bufs=3) as lab_pool,
            tc.tile_pool(name="pin", bufs=3) as p_pool,
            tc.tile_pool(name="tmp", bufs=3) as tmp_pool,
            tc.tile_pool(name="small", bufs=1) as small,
        ):
            iota_t = persist.tile([RPC, L], i32)
            nc.gpsimd.iota(
                iota_t[:], pattern=[[1, L]], base=1, channel_multiplier=0
            )

            c_full = persist.tile([RPC, L], f32)
            s_full = persist.tile([RPC, L], f32)
            lp_full = persist.tile([RPC, L], f32)
            e_full = persist.tile([RPC, L], f32)
            Zc = small.tile([RPC, NCH], f32)
            Ac = small.tile([RPC, NCH], f32)
            Bc = small.tile([RPC, NCH], f32)

            # Phase 1: stream in; running cumsum of labels; log(p).
            for j in range(NCH):
                sl = slice(j * CH, (j + 1) * CH)
                lab = lab_pool.tile([RPC, CH], i32, tag="lab")
                nc.sync.dma_start(lab[:], labels_d[:, sl])
                init = 0.0 if j == 0 else c_full[:, j * CH - 1 : j * CH]
                nc.vector.tensor_tensor_scan(
                    c_full[:, sl], lab[:], lab[:], init, Alu.add, Alu.bypass
                )
                pt = p_pool.tile([RPC, CH], f32, tag="p")
                nc.sync.dma_start(pt[:], p_d[:, sl])
                nc.scalar.activation(lp_full[:, sl], pt[:], Act.Ln)

            T_ap = c_full[:, L - 1 : L]
            # Phase 2: s = (2/tau)*c/(k+T); e = exp(s) with per-chunk Z.
            for j in range(NCH):
                sl = slice(j * CH, (j + 1) * CH)
                kT = tmp_pool.tile([RPC, CH], f32, tag="kT")
                nc.vector.tensor_scalar(kT[:], iota_t[:, sl], T_ap, None, Alu.add)
                inv = tmp_pool.tile([RPC, CH], f32, tag="inv")
                nc.vector.reciprocal_approx_fast(inv[:], kT[:])
                nc.vector.scalar_tensor_tensor(
                    s_full[:, sl],
                    c_full[:, sl],
                    2.0 / TAU,
                    inv[:],
                    Alu.mult,
                    Alu.mult,
                )
                nc.scalar.activation(
                    e_full[:, sl],
                    s_full[:, sl],
                    Act.Exp,
                    accum_out=Zc[:, j : j + 1],
                )

            Z = small.tile([RPC, 1], f32)
            nc.vector.tensor_reduce(Z[:], Zc[:], Ax.X, Alu.add)
            invZ = small.tile([RPC, 1], f32)
            nc.vector.reciprocal_approx_fast(invZ[:], Z[:])
            lnZ = small.tile([RPC, 1], f32)
            nc.scalar.activation(lnZ[:], Z[:], Act.Ln)

            # Phase 3: per-row A = sum q*s, B = sum q*ln p via fused
            # (in0*invZ)*in1 with free accumulate.
            for j in range(NCH):
                sl = slice(j * CH, (j + 1) * CH)
                scr_a = tmp_pool.tile([RPC, CH], f32, tag="scra")
                nc.vector.affine_mul_reduce(
                    scr_a[:],
                    Ac[:, j : j + 1],
                    s_full[:, sl],
                    e_full[:, sl],
                    invZ[:],
                    0.0,
                )
                scr_b = tmp_pool.tile([RPC, CH], f32, tag="scrb")
                nc.vector.affine_mul_reduce(
                    scr_b[:],
                    Bc[:, j : j + 1],
                    lp_full[:, sl],
                    e_full[:, sl],
                    invZ[:],
                    0.0,
                )

            A = small.tile([RPC, 1], f32)
            nc.vector.tensor_reduce(A[:], Ac[:], Ax.X, Alu.add)
            Bv = small.tile([RPC, 1], f32)
            nc.vector.tensor_reduce(Bv[:], Bc[:], Ax.X, Alu.add)
            u = small.tile([RPC, 1], f32)
            nc.vector.scalar_tensor_tensor(
                u[:], A[:], lnZ[:], Bv[:], Alu.subtract, Alu.subtract
            )
            res = small.tile([RPC, 1], f32)
            nc.gpsimd.partition_all_reduce(
                res[:], u[:], RPC, bass_isa.ReduceOp.add
            )
            nc.sync.dma_start(out_d[:, :], res[0:1, :])
    nc.compile()
    return nc


def get_nc():
    nc = _NC_CACHE.get("nc")
    if nc is None:
        nc = build_nc()
        _NC_CACHE["nc"] = nc
    return nc


def shard_inputs(output, labels):
    p = np.ascontiguousarray(
        np.asarray(output, dtype=np.float32).reshape(B, L)
    )
    lab = np.ascontiguousarray(np.asarray(labels, dtype=np.int32))
    return [
        {
            "labels": lab[i * RPC : (i + 1) * RPC],
            "p": p[i * RPC : (i + 1) * RPC],
        }
        for i in range(N_CORES)
    ]


def gather(results):
    total = np.float64(0.0)
    for r in results:
        total += np.float64(r["partial"].reshape(-1)[0])
    return np.array(total / B, dtype=np.float32)


def kernel(output, labels):
    from concourse.bass_utils import run_bass_kernel_spmd

    nc = get_nc()
    in_maps = shard_inputs(output, labels)
    res = run_bass_kernel_spmd(nc, in_maps, list(range(N_CORES)))
    return gather(res.results)


# revision 5
# speedup vs baseline: 1.1527x; 1.1527x over previous
"""KLDivLoss(batchmean) of softmax(f1_rewards/tau) against log(output).

Contract: kernel(output=[1024,4096,1] f32, labels=[1024,4096] i32) -> () f32.

Math (per batch row):
    c_k  = cumsum(labels)            (k = 1..L)
    T    = c_L
    r_k  = 2*c_k / (k + T)           (== F1@k; the where() guards in the
                                      reference collapse since c_k = 0 => r_k = 0)
    s_k  = r_k / tau                 (s in [0, ~1.18] -> exp safe without
                                      max-subtraction)
    q    = softmax(s);  Z = sum exp(s);  log q = s - ln Z
    row  = sum_k q_k*s_k - ln Z - sum_k q_k*ln p_k
    loss = sum_rows(row) / B

Distribution: pure data-parallel, 128 batch rows per NeuronCore (= the 128
SBUF partitions), 8 cores. Each core emits one f32 partial (its row-sum);
the host adds the 8 partials and divides by B.

Engine split per core:
    DVE    : cumsum scan (chunked, chained carry), kT = iota + T,
             reciprocal_approx_fast, s = (2/tau)*c*inv, e' = e*invZ
    ACT    : ln(p), exp(s) with free per-chunk Z accumulate, ln(Z)
    PE     : the row-dot contractions sum_f q*s and sum_f q*ln p via
             accumulated diagonal-block matmuls in fp16 (PSUM fp32)
    GPSIMD : iota constant, final partition reduce
"""

import numpy as np

B, L = 1024, 4096
N_CORES = 8
RPC = B // N_CORES  # rows per core = 128 = SBUF partitions
TAU = 0.85
CH = 1024  # free-dim chunk
NCH = L // CH
MM = 128  # matmul window

_NC_CACHE = {}


def build_nc():
    import concourse.bacc as bacc
    import concourse.bass_isa as bass_isa
    import concourse.mybir as mybir
    import concourse.tile as tile

    f32 = mybir.dt.float32
    f16 = mybir.dt.float16
    i32 = mybir.dt.int32
    Alu = mybir.AluOpType
    Act = mybir.ActivationFunctionType
    Ax = mybir.AxisListType

    nc = bacc.Bacc(
        "TRN2", target_bir_lowering=False, debug=False, num_devices=N_CORES
    )
    labels_d = nc.dram_tensor("labels", [RPC, L], i32, kind="ExternalInput").ap()
    p_d = nc.dram_tensor("p", [RPC, L], f32, kind="ExternalInput").ap()
    out_d = nc.dram_tensor("partial", [1, 1], f32, kind="ExternalOutput").ap()

    with tile.TileContext(nc) as tc:
        with (
            tc.tile_pool(name="persist", bufs=1) as persist,
            tc.tile_pool(name="lab", bufs=3) as lab_pool,
            tc.tile_pool(name="pin", bufs=3) as p_pool,
            tc.tile_pool(name="tmp", bufs=2) as tmp_pool,
            tc.tile_pool(name="small", bufs=1) as small,
            tc.tile_pool(name="psum", bufs=1, space="PSUM") as psum_pool,
        ):
            iota_t = persist.tile([RPC, L], i32)
            nc.gpsimd.iota(
                iota_t[:], pattern=[[1, L]], base=1, channel_multiplier=0
            )
            # identity matrix for extracting the diagonal of PSUM blocks
            ident = persist.tile([MM, MM], f32)
            nc.gpsimd.memset(ident[:], 1.0)
            nc.gpsimd.affine_select(
                ident[:],
                ident[:],
                pattern=[[-1, MM]],
                compare_op=Alu.is_equal,
                fill=0.0,
                base=0,
                channel_multiplier=1,
            )

            c_full = persist.tile([RPC, L], f32)
            s_full = persist.tile([RPC, L], f16)
            lp_full = persist.tile([RPC, L], f16)
            e_full = persist.tile([RPC, L], f32)
            Zc = small.tile([RPC, NCH], f32)

            # Phase 1: stream in; running cumsum of labels; ln(p) (fp16 out).
            for j in range(NCH):
                sl = slice(j * CH, (j + 1) * CH)
                lab = lab_pool.tile([RPC, CH], i32, tag="lab")
                nc.sync.dma_start(lab[:], labels_d[:, sl])
                init = 0.0 if j == 0 else c_full[:, j * CH - 1 : j * CH]
                nc.vector.tensor_tensor_scan(
                    c_full[:, sl], lab[:], lab[:], init, Alu.add, Alu.bypass
                )
                pt = p_pool.tile([RPC, CH], f32, tag="p")
                nc.sync.dma_start(pt[:], p_d[:, sl])
                nc.scalar.activation(lp_full[:, sl], pt[:], Act.Ln)

            T_ap = c_full[:, L - 1 : L]
            # Phase 2: s = (2/tau)*c/(k+T) (fp16 out); e = exp(s), chunk Z.
            for j in range(NCH):
                sl = slice(j * CH, (j + 1) * CH)
                kT = tmp_pool.tile([RPC, CH], f32, tag="kT")
                nc.vector.tensor_scalar(kT[:], iota_t[:, sl], T_ap, None, Alu.add)
                inv = tmp_pool.tile([RPC, CH], f32, tag="inv")
                nc.vector.reciprocal_approx_fast(inv[:], kT[:])
                nc.vector.scalar_tensor_tensor(
                    s_full[:, sl],
                    c_full[:, sl],
                    2.0 / TAU,
                    inv[:],
                    Alu.mult,
                    Alu.mult,
                )
                nc.scalar.activation(
                    e_full[:, sl],
                    s_full[:, sl],
                    Act.Exp,
                    accum_out=Zc[:, j : j + 1],
                )

            Z = small.tile([RPC, 1], f32)
            nc.vector.tensor_reduce(Z[:], Zc[:], Ax.X, Alu.add)
            invZ = small.tile([RPC, 1], f32)
            nc.vector.reciprocal_approx_fast(invZ[:], Z[:])
            lnZ = small.tile([RPC, 1], f32)
            nc.scalar.activation(lnZ[:], Z[:], Act.Ln)

            # Phase 3: q = e*invZ (fp16); PE accumulates the diagonal-block
            # products q (x) s and q (x) lp over all 128-wide windows.
            psum_a = psum_pool.tile([MM, MM], f32, tag="pa")
            psum_b = psum_pool.tile([MM, MM], f32, tag="pb")
            nwin = L // MM
            for j in range(NCH):
                sl = slice(j * CH, (j + 1) * CH)
                ep = tmp_pool.tile([RPC, CH], f16, tag="ep")
                nc.vector.tensor_scalar(
                    ep[:], e_full[:, sl], invZ[:], None, Alu.mult
                )
                for w in range(CH // MM):
                    g = j * (CH // MM) + w
                    wsl = slice(w * MM, (w + 1) * MM)
                    nc.tensor.matmul(
                        psum_a[:],
                        ep[:, wsl],
                        s_full[:, j * CH + w * MM : j * CH + (w + 1) * MM],
                        start=(g == 0),
                        stop=(g == nwin - 1),
                    )
                    nc.tensor.matmul(
                        psum_b[:],
                        ep[:, wsl],
                        lp_full[:, j * CH + w * MM : j * CH + (w + 1) * MM],
                        start=(g == 0),
                        stop=(g == nwin - 1),
                    )

            scr_a = small.tile([MM, MM], f32)
            diag_a = small.tile([MM, 1], f32)
            nc.vector.scalar_tensor_tensor(
                scr_a[:], psum_a[:], 1.0, ident[:], Alu.mult, Alu.mult,
                accum_out=diag_a[:],
            )
            scr_b = small.tile([MM, MM], f32)
            diag_b = small.tile([MM, 1], f32)
            nc.vector.scalar_tensor_tensor(
                scr_b[:], psum_b[:], 1.0, ident[:], Alu.mult, Alu.mult,
                accum_out=diag_b[:],
            )

            u = small.tile([RPC, 1], f32)
            nc.vector.scalar_tensor_tensor(
                u[:], diag_a[:], lnZ[:], diag_b[:], Alu.subtract, Alu.subtract
            )
            res = small.tile([RPC, 1], f32)
            nc.gpsimd.partition_all_reduce(
                res[:], u[:], RPC, bass_isa.ReduceOp.add
            )
            nc.sync.dma_start(out_d[:, :], res[0:1, :])
    nc.compile()
    return nc


def get_nc():
    nc = _NC_CACHE.get("nc")
    if nc is None:
        nc = build_nc()
        _NC_CACHE["nc"] = nc
    return nc


def shard_inputs(output, labels):
    p = np.ascontiguousarray(
        np.asarray(output, dtype=np.float32).reshape(B, L)
    )
    lab = np.ascontiguousarray(np.asarray(labels, dtype=np.int32))
    return [
        {
            "labels": lab[i * RPC : (i + 1) * RPC],
            "p": p[i * RPC : (i + 1) * RPC],
        }
        for i in range(N_CORES)
    ]


def gather(results):
    total = np.float64(0.0)
    for r in results:
        total += np.float64(r["partial"].reshape(-1)[0])
    return np.array(total / B, dtype=np.float32)


def kernel(output, labels):
    from concourse.bass_utils import run_bass_kernel_spmd

    nc = get_nc()
    in_maps = shard_inputs(output, labels)
    res = run_bass_kernel_spmd(nc, in_maps, list(range(N_CORES)))
    return gather(res.results)
